# revision 1
# baseline (speedup 1.0000x reference)
"""Trainium2 Bass kernel for nn_DSR_GCN (dual-superpixel GCN).

Sharding (8 NeuronCores, SPMD): row-shard the HW=65536 pixel dim (8192
rows/core).  Pass 1 computes per-core partials G.T = x_shard.T @ Q_shard and
column sums (ones-stationary matmuls sharing the same moving Q stream), then
AllReduces them (big branch early so its GCN overlaps the small pass-1).
The small [N,N] GCN math is replicated on every core in "transposed land"
(feature-major [F, N] layouts) so BatchNorm/bias are per-partition ops.
Pass 2 computes z.T = RP1.T @ Q.T + RP2.T @ Qs.T with the final linear
layers folded into tiny [N,32] bf16 stationaries, transposes 512-row chunks
back to pixel-major via the PE, and runs the softmax/loss epilogue.
Heavy matmul streams are bf16 (host-cast); on-chip f32 matmuls use f32r
where the moving dim >= 256.
"""

import os
import numpy as np
import ml_dtypes

BF16 = ml_dtypes.bfloat16

HW, C = 65536, 128
NB, NS, NCLS = 1024, 2048, 16
NCORES = 8
EPS = 1e-5
CLAMP = 0.03


def _emit(nc, tc, ctx, rows, nb, ns, ncores):
    import concourse.bass as bass
    import concourse.mybir as mybir
    from concourse import masks
    from contextlib import ExitStack

    f32 = mybir.dt.float32
    f32r = mybir.dt.float32r
    bf16 = mybir.dt.bfloat16
    ts = bass.ts
    AF = mybir.ActivationFunctionType
    ALU = mybir.AluOpType
    AX = mybir.AxisListType.X

    def r32(ap):
        return ap.bitcast(f32r)

    # ---- dram I/O ----
    din = lambda n_, s, d: nc.dram_tensor(n_, s, d, kind="ExternalInput")
    xs = din("xs", [rows, C], bf16)
    q = din("q", [rows, nb], bf16)
    qs = din("qs", [rows, ns], bf16)
    qbt = din("qbt", [nb, rows], bf16)
    qst = din("qst", [ns, rows], bf16)
    at = din("at", [nb, nb], bf16)
    ast = din("ast", [ns, ns], bf16)
    yt = din("yt", [64, rows], f32)
    FOS = [128, 64, 128, 64]
    wls = [din(f"wl{i}", [128, 256 + 2 * fo + 5], f32) for i, fo in enumerate(FOS)]
    misc = din("misc", [64, 81], f32)
    yo = nc.dram_tensor("yo", [rows, NCLS], f32, kind="ExternalOutput")
    lo = nc.dram_tensor("lo", [rows, NCLS], f32, kind="ExternalOutput")

    # ---- persistent pools ----
    consts = ctx.enter_context(tc.tile_pool(name="consts", bufs=1))
    gwork = ctx.enter_context(tc.tile_pool(name="gwork", bufs=1))
    dram = ctx.enter_context(tc.tile_pool(name="dram", bufs=1, space="DRAM"))

    ident32 = consts.tile([32, 32], f32)
    masks.make_identity(nc, ident32[:])
    ident1 = consts.tile([1, 1], f32)
    nc.gpsimd.memset(ident1[:], 1.0)
    ones_k1 = consts.tile([1, 128], f32)
    nc.gpsimd.memset(ones_k1[:], 1.0)
    ones_bf = consts.tile([128, 1], bf16)
    nc.gpsimd.memset(ones_bf[:], 1.0)
    eps_c = consts.tile([128, 1], f32)
    nc.gpsimd.memset(eps_c[:], EPS)

    misc_sb = consts.tile([64, 81], f32)
    nc.gpsimd.dma_start(misc_sb[:], misc[:])
    wl_sb = []
    for i, fo in enumerate(FOS):
        t = consts.tile([128, 256 + 2 * fo + 5], f32, tag=f"wl{i}")
        nc.gpsimd.dma_start(t[:], wls[i][:])
        wl_sb.append(t)

    # ---- pass 1 ----
    n_rt = rows // 128
    shkw = {"addr_space": "Shared"} if ncores > 4 else {}
    ar1_in = dram.tile([129, nb], f32, tag="ar1i")
    ar1_out = dram.tile([129, nb], f32, tag="ar1o", **shkw)
    ar2_in = dram.tile([129, ns], f32, tag="ar2i")
    ar2_out = dram.tile([129, ns], f32, tag="ar2o", **shkw)

    with tc.tile_pool(name="p1pool", bufs=1) as p1pool:
        xall = p1pool.tile([128, n_rt * C], bf16, tag="xall")
        nc.gpsimd.dma_start(
            xall[:].rearrange("p (t c) -> p t c", c=C),
            xs[:].rearrange("(t p) c -> p t c", p=128))

        def pass1_phase(qd, n, g_ps, cs_ps, rgrp, qtag, qpool):
            for g in range(n_rt // rgrp):
                qt = qpool.tile([128, rgrp * n], bf16, tag=qtag)
                for a in range(rgrp):
                    rt = g * rgrp + a
                    nc.gpsimd.dma_start(qt[:, a * n:(a + 1) * n],
                                        qd[rt * 128:(rt + 1) * 128, :])
                for a in range(rgrp):
                    rt = g * rgrp + a
                    xt = xall[:, ts(rt, C)]
                    st = (rt == 0)
                    sp = (rt == n_rt - 1)
                    for cnk in range(n // 512):
                        mv = qt[:, a * n + cnk * 512:a * n + (cnk + 1) * 512]
                        nc.tensor.matmul(g_ps[:, ts(cnk, 512)], xt, mv,
                                         start=st, stop=sp)
                        nc.tensor.matmul(cs_ps[:, ts(cnk, 512)],
                                         ones_bf[:], mv, start=st, stop=sp)

        with tc.tile_pool(name="ps_p1b", bufs=1, space="PSUM") as psb, \
             tc.tile_pool(name="qpb", bufs=3) as qpool:
            g1p = psb.tile([128, nb], f32, tag="g1p")
            cs1p = psb.tile([1, nb], f32, tag="cs1p")
            pass1_phase(q, nb, g1p, cs1p, min(4096 // nb, n_rt), "qb", qpool)
            g1t = p1pool.tile([128, nb], f32, tag="g1t")
            cs1 = p1pool.tile([1, nb], f32, tag="cs1")
            nc.vector.tensor_copy(g1t[:], g1p[:])
            nc.vector.tensor_copy(cs1[:], cs1p[:])

        # big-branch AllReduce early: overlaps small pass-1
        nc.gpsimd.dma_start(ar1_in[0:128, :], g1t[:])
        nc.gpsimd.dma_start(ar1_in[128:129, :], cs1[:])
        nc.gpsimd.collective_compute(
            "AllReduce", mybir.AluOpType.add,
            replica_groups=[list(range(ncores))],
            ins=[ar1_in.opt()], outs=[ar1_out.opt()])

        with tc.tile_pool(name="ps_p1s", bufs=1, space="PSUM") as pss, \
             tc.tile_pool(name="qps", bufs=3) as qpool:
            g2p = pss.tile([128, ns], f32, tag="g2p")
            cs2p = pss.tile([1, ns], f32, tag="cs2p")
            pass1_phase(qs, ns, g2p, cs2p, min(4096 // ns, n_rt), "qs", qpool)
            g2t = p1pool.tile([128, ns], f32, tag="g2t")
            cs2 = p1pool.tile([1, ns], f32, tag="cs2")
            nc.vector.tensor_copy(g2t[:], g2p[:])
            nc.vector.tensor_copy(cs2[:], cs2p[:])

        nc.gpsimd.dma_start(ar2_in[0:128, :], g2t[:])
        nc.gpsimd.dma_start(ar2_in[128:129, :], cs2[:])
        nc.gpsimd.collective_compute(
            "AllReduce", mybir.AluOpType.add,
            replica_groups=[list(range(ncores))],
            ins=[ar2_in.opt()], outs=[ar2_out.opt()])

    # ---- GCN (replicated per core) ----
    def gcn_branch(n, ar_out, at_d, lidx, clamp, btag, hfin):
        njt = n // 128
        ncnk = n // 512
        with ExitStack() as bctx:
            bp = bctx.enter_context(tc.tile_pool(name=f"b_{btag}", bufs=1))

            ht = bp.tile([128, n], f32, tag="hcur0")
            with tc.tile_pool(name=f"psr_{btag}", bufs=2, space="PSUM") as psr, \
                 tc.tile_pool(name=f"icsp_{btag}", bufs=1) as icsp:
                g_sb = icsp.tile([128, n], f32, tag="g_sb")
                cs_sb = icsp.tile([1, n], f32, tag="cs_sb")
                nc.gpsimd.dma_start(g_sb[:], ar_out[0:128, :])
                nc.gpsimd.dma_start(cs_sb[:], ar_out[128:129, :])
                ics = icsp.tile([1, n], f32, tag="ics")
                nc.vector.reciprocal(ics[:], cs_sb[:])
                for cnk in range(ncnk):
                    pr = psr.tile([128, 512], f32)
                    nc.tensor.matmul(pr[:], ones_k1[:],
                                     ics[:, ts(cnk, 512)],
                                     start=True, stop=True)
                    nc.vector.tensor_tensor(
                        ht[:, ts(cnk, 512)], g_sb[:, ts(cnk, 512)], pr[:],
                        op=ALU.mult)

            for li2, wli in enumerate(lidx):
                fo = FOS[wli]
                wl = wl_sb[wli]
                last = (li2 == 1)
                c0 = 256 + fo
                thWT = wl[:, 0:256]
                oWT = wl[:, 256:256 + fo]
                bng = wl[:, c0:c0 + 1]
                bnb = wl[:, c0 + 1:c0 + 2]
                thb = [wl[:, c0 + 2:c0 + 3], wl[:, c0 + 3:c0 + 4]]
                ob_col = wl[0:fo, c0 + 4:c0 + 5]
                ob_row = wl[0:1, c0 + 5:c0 + 5 + fo]

                with ExitStack() as lctx:
                    lp = lctx.enter_context(
                        tc.tile_pool(name=f"l_{btag}{li2}", bufs=1))
                    sp = lctx.enter_context(
                        tc.tile_pool(name=f"sp_{btag}{li2}", bufs=2))

                    # --- batchnorm over nodes (free dim) ---
                    s1 = sp.tile([128, 1], f32, tag="s1")
                    nc.vector.reduce_sum(out=s1[:], in_=ht[:], axis=AX)
                    s2p = sp.tile([128, ncnk], f32, tag="s2p")
                    with tc.tile_pool(name=f"psbn_{btag}{li2}", bufs=2,
                                      space="PSUM") as psbn:
                        for cnk in range(ncnk):
                            pb = psbn.tile([128, 512], f32)
                            nc.scalar.activation(
                                pb[:], ht[:, ts(cnk, 512)], AF.Square,
                                accum_out=s2p[:, cnk:cnk + 1])
                    s2 = sp.tile([128, 1], f32, tag="s2")
                    nc.vector.reduce_sum(out=s2[:], in_=s2p[:], axis=AX)
                    m = sp.tile([128, 1], f32, tag="m")
                    nc.vector.tensor_scalar_mul(m[:], s1[:], 1.0 / n)
                    v = sp.tile([128, 1], f32, tag="v")
                    nc.vector.tensor_scalar_mul(v[:], s2[:], 1.0 / n)
                    m2 = sp.tile([128, 1], f32, tag="m2")
                    nc.vector.tensor_tensor(m2[:], m[:], m[:], op=ALU.mult)
                    nc.vector.tensor_tensor(v[:], v[:], m2[:], op=ALU.subtract)
                    sd = sp.tile([128, 1], f32, tag="sd")
                    nc.scalar.activation(sd[:], v[:], AF.Sqrt, bias=eps_c[:])
                    isd = sp.tile([128, 1], f32, tag="isd")
                    nc.vector.reciprocal(isd[:], sd[:])
                    kk = sp.tile([128, 1], f32, tag="kk")
                    nc.vector.tensor_tensor(kk[:], bng, isd[:], op=ALU.mult)
                    b2 = sp.tile([128, 1], f32, tag="b2")
                    nc.vector.tensor_tensor(b2[:], m[:], kk[:], op=ALU.mult)
                    nc.vector.tensor_tensor(b2[:], bnb, b2[:], op=ALU.subtract)
                    hbn = lp.tile([128, n], bf16, tag="hbn")
                    nc.vector.tensor_scalar(hbn[:], ht[:], kk[:], b2[:],
                                            op0=ALU.mult, op1=ALU.add)
                    thWT_bf = lp.tile([128, 256], bf16, tag="thWT_bf")
                    nc.vector.tensor_copy(thWT_bf[:], thWT)
                    oWT_bf = lp.tile([128, fo], bf16, tag="oWT_bf")
                    nc.vector.tensor_copy(oWT_bf[:], oWT)

                    # --- Hx.T = thW @ Hbn.T + thb ---
                    hx = [lp.tile([128, n], bf16, tag=f"hx{k}", name=f"hx{k}")
                          for k in range(2)]
                    with tc.tile_pool(name=f"psx_{btag}{li2}", bufs=3,
                                      space="PSUM") as psx:
                        for k in range(2):
                            for cnk in range(ncnk):
                                px = psx.tile([128, 512], f32)
                                nc.tensor.matmul(
                                    px[:], thWT_bf[:, ts(k, 128)],
                                    hbn[:, ts(cnk, 512)],
                                    start=True, stop=True)
                                nc.vector.tensor_scalar_add(
                                    hx[k][:, ts(cnk, 512)], px[:], thb[k])

                    # --- S blocks -> sigmoid -> t = S'*A.T ; d_pre ---
                    tt = []
                    thr = float(np.log(CLAMP / (1.0 - CLAMP)))
                    with tc.tile_pool(name=f"psd_{btag}{li2}", bufs=1,
                                      space="PSUM") as psd, \
                         tc.tile_pool(name=f"pss_{btag}{li2}", bufs=3,
                                      space="PSUM") as pssb, \
                         tc.tile_pool(name=f"atp_{btag}{li2}", bufs=2) as atp:
                        dpre_ps = psd.tile([1, n], f32, tag="dpre")
                        for j in range(njt):
                            sbl = sp.tile([128, n], bf16, tag="sblk")
                            for cnk in range(ncnk):
                                px = pssb.tile([128, 512], f32)
                                nc.tensor.matmul(px[:],
                                                 hx[0][:, ts(j, 128)],
                                                 hx[0][:, ts(cnk, 512)],
                                                 start=True, stop=False)
                                nc.tensor.matmul(px[:],
                                                 hx[1][:, ts(j, 128)],
                                                 hx[1][:, ts(cnk, 512)],
                                                 start=False, stop=True)
                                if clamp:
                                    xc = sp.tile([128, 512], f32, tag="xc")
                                    nc.vector.tensor_scalar_max(
                                        xc[:], px[:], thr)
                                    nc.scalar.activation(
                                        sbl[:, ts(cnk, 512)], xc[:], AF.Sigmoid)
                                else:
                                    nc.scalar.activation(
                                        sbl[:, ts(cnk, 512)], px[:], AF.Sigmoid)
                            att = atp.tile([128, n], bf16)
                            nc.gpsimd.dma_start(att[:], at_d[ts(j, 128), :])
                            tj = lp.tile([128, n], bf16, tag=f"tj{j}")
                            nc.vector.tensor_tensor(tj[:], sbl[:], att[:],
                                                    op=ALU.mult)
                            tt.append(tj)
                            for cnk in range(ncnk):
                                nc.tensor.matmul(
                                    dpre_ps[:, ts(cnk, 512)], ones_bf[:],
                                    tj[:, ts(cnk, 512)],
                                    start=(j == 0), stop=(j == njt - 1))
                        dpre = lp.tile([1, n], f32, tag="dpre_sb")
                        nc.vector.tensor_copy(dpre[:], dpre_ps[:])

                    # d = (dpre + 1)^-1/2
                    drow = lp.tile([1, n], f32, tag="drow")
                    nc.scalar.activation(dpre[:], dpre[:], AF.Sqrt,
                                         bias=ident1[:])
                    nc.vector.reciprocal(drow[:], dpre[:])

                    # d as per-partition columns (PE transpose 128-blocks)
                    dcol = sp.tile([128, njt], f32, tag="dcol")
                    ob_rep = sp.tile([128, fo], f32, tag="ob_rep")
                    with tc.tile_pool(name=f"pst_{btag}{li2}", bufs=3,
                                      space="PSUM") as pst:
                        for j in range(njt):
                            pt = pst.tile([128, 1], f32, tag="dt")
                            nc.tensor.transpose(pt[:], drow[:, ts(j, 128)],
                                                ident1[:])
                            nc.vector.tensor_copy(dcol[:, j:j + 1], pt[:])
                        pr = pst.tile([128, fo], f32, tag="obr")
                        nc.tensor.matmul(pr[:], ones_k1[:], ob_row,
                                         start=True, stop=True)
                        nc.vector.tensor_copy(ob_rep[:], pr[:])

                    # stat_j = d_j * (HoW_j + ob)   [128, fo] bf16
                    stats = []
                    with tc.tile_pool(name=f"psh_{btag}{li2}", bufs=3,
                                      space="PSUM") as psh:
                        for j in range(njt):
                            ph = psh.tile([128, fo], f32)
                            nc.tensor.matmul(ph[:], hbn[:, ts(j, 128)],
                                             oWT_bf[:], start=True, stop=True)
                            w1 = sp.tile([128, fo], f32, tag="w1")
                            nc.vector.tensor_tensor(w1[:], ph[:], ob_rep[:],
                                                    op=ALU.add)
                            stj = lp.tile([128, fo], bf16, tag=f"st{j}")
                            nc.vector.tensor_scalar_mul(stj[:], w1[:],
                                                        dcol[:, j:j + 1])
                            stats.append(stj)

                    # u = HoW.T + ob
                    u = lp.tile([fo, n], f32, tag="u")
                    with tc.tile_pool(name=f"psu_{btag}{li2}", bufs=2,
                                      space="PSUM") as psu:
                        for cnk in range(ncnk):
                            pu = psu.tile([fo, 512], f32)
                            nc.tensor.matmul(pu[:], oWT_bf[:],
                                             hbn[:, ts(cnk, 512)],
                                             start=True, stop=True)
                            nc.vector.tensor_scalar_add(
                                u[:, ts(cnk, 512)], pu[:], ob_col)

                    # out.T = d ⊙ (stat.T @ t + d ⊙ u); leaky relu
                    hnext = hfin if last else bp.tile([128, n], f32,
                                                      tag="hcur1")
                    with tc.tile_pool(name=f"pso_{btag}{li2}", bufs=1,
                                      space="PSUM") as pso, \
                         tc.tile_pool(name=f"psq_{btag}{li2}", bufs=2,
                                      space="PSUM") as psq:
                        po = pso.tile([fo, n], f32, tag="po")
                        for j in range(njt):
                            for cnk in range(ncnk):
                                nc.tensor.matmul(po[:, ts(cnk, 512)],
                                                 stats[j][:],
                                                 tt[j][:, ts(cnk, 512)],
                                                 start=(j == 0),
                                                 stop=(j == njt - 1))
                        for cnk in range(ncnk):
                            pr = psq.tile([fo, 512], f32)
                            nc.tensor.matmul(pr[:], ones_k1[:, 0:fo],
                                             drow[:, ts(cnk, 512)],
                                             start=True, stop=True)
                            z1 = sp.tile([fo, 512], f32, tag="z1")
                            nc.vector.tensor_tensor(
                                z1[:], u[:, ts(cnk, 512)], pr[:], op=ALU.mult)
                            vv = sp.tile([fo, 512], f32, tag="vv")
                            nc.vector.tensor_tensor(
                                vv[:], po[:, ts(cnk, 512)], z1[:], op=ALU.add)
                            nc.vector.tensor_tensor(vv[:], vv[:], pr[:],
                                                    op=ALU.mult)
                            lk = sp.tile([fo, 512], f32, tag="lk")
                            nc.vector.tensor_scalar_mul(lk[:], vv[:], 0.01)
                            nc.vector.tensor_tensor(
                                hnext[0:fo, ts(cnk, 512)], vv[:], lk[:],
                                op=ALU.max)
                ht = hnext

    h1f = gwork.tile([64, nb], f32, tag="h1f")
    h2f = gwork.tile([64, ns], f32, tag="h2f")
    gcn_branch(nb, ar1_out, at, [0, 1], True, "big", h1f)
    gcn_branch(ns, ar2_out, ast, [2, 3], False, "sml", h2f)

    # RP stationaries [spix, 32] bf16, final linears folded
    wcb = misc_sb[:, 0:32]
    wcs = misc_sb[:, 32:64]
    w128bT = misc_sb[:, 64:80]
    b128 = misc_sb[0:16, 80:81]
    rp1 = gwork.tile([128, (nb // 128) * 32], bf16, tag="rp1")
    rp2 = gwork.tile([128, (ns // 128) * 32], bf16, tag="rp2")
    with tc.tile_pool(name="psrp", bufs=3, space="PSUM") as psrp:
        for j in range(nb // 128):
            pr = psrp.tile([128, 32], f32)
            nc.tensor.matmul(pr[:], h1f[:, ts(j, 128)], wcb,
                             start=True, stop=True)
            nc.vector.tensor_copy(rp1[:, ts(j, 32)], pr[:])
        for j in range(ns // 128):
            pr = psrp.tile([128, 32], f32)
            nc.tensor.matmul(pr[:], h2f[:, ts(j, 128)], wcs,
                             start=True, stop=True)
            nc.vector.tensor_copy(rp2[:, ts(j, 32)], pr[:])

    # ---- pass 2 + epilogue ----
    GRP = min(2048, rows)
    nrc = max(GRP // 512, 1)
    CH = GRP // nrc
    with tc.tile_pool(name="qtp", bufs=6) as qtp, \
         tc.tile_pool(name="ytp", bufs=2) as ytp, \
         tc.tile_pool(name="ps_z", bufs=4, space="PSUM") as ps_z, \
         tc.tile_pool(name="ps_yw", bufs=2, space="PSUM") as ps_yw, \
         tc.tile_pool(name="ps_tp", bufs=2, space="PSUM") as ps_tp, \
         tc.tile_pool(name="epil", bufs=4) as ep:
        for gidx in range(rows // GRP):
            ytt = ytp.tile([64, GRP], f32, tag="ytt")
            nc.gpsimd.dma_start(ytt[:], yt[:, gidx * GRP:(gidx + 1) * GRP])
            pz = [ps_z.tile([32, CH], f32, tag="pz", name=f"pz{gidx}_{i}")
                  for i in range(nrc)]
            for j in range(nb // 128):
                tq = qtp.tile([128, GRP], bf16, tag="tqb")
                nc.gpsimd.dma_start(
                    tq[:], qbt[ts(j, 128), gidx * GRP:(gidx + 1) * GRP])
                for rc in range(nrc):
                    nc.tensor.matmul(pz[rc][:], rp1[:, ts(j, 32)],
                                     tq[:, ts(rc, CH)],
                                     start=(j == 0), stop=False)
            for j in range(ns // 128):
                tq = qtp.tile([128, GRP], bf16, tag="tqs")
                nc.gpsimd.dma_start(
                    tq[:], qst[ts(j, 128), gidx * GRP:(gidx + 1) * GRP])
                for rc in range(nrc):
                    nc.tensor.matmul(pz[rc][:], rp2[:, ts(j, 32)],
                                     tq[:, ts(rc, CH)],
                                     start=False, stop=(j == ns // 128 - 1))
            for rc in range(nrc):
                base = gidx * GRP + rc * CH
                pyw = ps_yw.tile([16, CH], f32)
                nc.tensor.matmul(pyw[:], w128bT, ytt[:, ts(rc, CH)],
                                 start=True, stop=True)
                yws = ep.tile([16, CH], f32, tag="yws")
                nc.scalar.activation(yws[:], pyw[:], AF.Copy)
                tri = ep.tile([32, CH], f32, tag="tri")
                nc.scalar.activation(tri[:], pz[rc][:], AF.Copy)
                nc.vector.scalar_tensor_tensor(
                    tri[0:16, :], pz[rc][0:16, :], b128, yws[:],
                    op0=ALU.add, op1=ALU.add)
                for s in range(CH // 128):
                    ptr = ps_tp.tile([128, 32], f32)
                    nc.tensor.transpose(ptr[:], tri[:, ts(s, 128)], ident32[:])
                    mx = ep.tile([128, 1], f32, tag="mx")
                    nc.vector.reduce_max(out=mx[:], in_=ptr[:, 0:16], axis=AX)
                    nmx = ep.tile([128, 1], f32, tag="nmx")
                    nc.vector.tensor_scalar_mul(nmx[:], mx[:], -1.0)
                    e = ep.tile([128, 16], f32, tag="e")
                    ssum = ep.tile([128, 1], f32, tag="ssum")
                    nc.scalar.activation(e[:], ptr[:, 0:16], AF.Exp,
                                         bias=nmx[:], accum_out=ssum[:])
                    rcp = ep.tile([128, 1], f32, tag="rcp")
                    nc.vector.reciprocal(rcp[:], ssum[:])
                    yot = ep.tile([128, 16], f32, tag="yot")
                    nc.vector.tensor_scalar_mul(yot[:], e[:], rcp[:])
                    lot = ep.tile([128, 16], f32, tag="lot")
                    nc.scalar.activation(lot[:], ptr[:, 16:32], AF.Square)
                    nc.gpsimd.dma_start(
                        yo[base + s * 128:base + (s + 1) * 128, :], yot[:])
                    nc.gpsimd.dma_start(
                        lo[base + s * 128:base + (s + 1) * 128, :], lot[:])


def build(rows=HW // NCORES, nb=NB, ns=NS, ncores=NCORES):
    from contextlib import ExitStack
    import concourse.bacc as bacc
    import concourse.tile as tile

    nc = bacc.Bacc("TRN2", target_bir_lowering=False, debug=False,
                   enable_asserts=True, num_devices=ncores)
    with tile.TileContext(nc) as tc:
        with ExitStack() as ctx:
            _emit(nc, tc, ctx, rows, nb, ns, ncores)
    nc.compile()
    return nc


# --------------------------------------------------------------------------
# host wrapper
# --------------------------------------------------------------------------

def prep_inputs(rows, nb, ns, ncores,
                x, y, Q, A, Qsmall, Asmall,
                b0_bng, b0_bnb, b0_thW, b0_thb, b0_oW, b0_ob,
                b1_bng, b1_bnb, b1_thW, b1_thb, b1_oW, b1_ob,
                s0_bng, s0_bnb, s0_thW, s0_thb, s0_oW, s0_ob,
                s1_bng, s1_bnb, s1_thW, s1_thb, s1_oW, s1_ob,
                lin128_W, lin128_b, lin64_W, lin64_b, sigma2):
    f = np.float32
    hw = rows * ncores
    flat = np.ascontiguousarray(np.asarray(x, f).reshape(hw, -1))
    Q = np.asarray(Q, f)
    Qs = np.asarray(Qsmall, f)
    y = np.asarray(y, f)

    def wl_pack(thW, thb, oW, ob, bng, bnb):
        fo = np.asarray(oW).shape[0]
        w = np.zeros((128, 256 + 2 * fo + 5), f)
        w[:, 0:256] = np.asarray(thW, f).T
        w[:, 256:256 + fo] = np.asarray(oW, f).T
        c0 = 256 + fo
        w[:, c0] = np.asarray(bng, f)
        w[:, c0 + 1] = np.asarray(bnb, f)
        w[:, c0 + 2] = np.asarray(thb, f)[0:128]
        w[:, c0 + 3] = np.asarray(thb, f)[128:256]
        w[0:fo, c0 + 4] = np.asarray(ob, f)
        w[0, c0 + 5:c0 + 5 + fo] = np.asarray(ob, f)
        return w

    wl = [
        wl_pack(b0_thW, b0_thb, b0_oW, b0_ob, b0_bng, b0_bnb),
        wl_pack(b1_thW, b1_thb, b1_oW, b1_ob, b1_bng, b1_bnb),
        wl_pack(s0_thW, s0_thb, s0_oW, s0_ob, s0_bng, s0_bnb),
        wl_pack(s1_thW, s1_thb, s1_oW, s1_ob, s1_bng, s1_bnb),
    ]

    sig = float(np.asarray(sigma2).reshape(-1)[0])
    W128 = np.asarray(lin128_W, f)
    W64 = np.asarray(lin64_W, f)
    misc = np.zeros((64, 81), f)
    misc[:, 0:16] = sig * W128[:, :64].T
    misc[:, 16:32] = W64.T
    misc[:, 32:48] = (1.0 - sig) * W128[:, :64].T
    misc[:, 48:64] = -W64.T
    misc[:, 64:80] = W128[:, 64:].T
    misc[0:16, 80] = np.asarray(lin128_b, f)

    at_b = np.ascontiguousarray(np.asarray(A, f).T).astype(BF16)
    ast_b = np.ascontiguousarray(np.asarray(Asmall, f).T).astype(BF16)

    in_maps = []
    for c in range(ncores):
        r0, r1 = c * rows, (c + 1) * rows
        qsh = Q[r0:r1]
        qssh = Qs[r0:r1]
        m = {
            "xs": flat[r0:r1].astype(BF16),
            "q": qsh.astype(BF16),
            "qs": qssh.astype(BF16),
            "qbt": np.ascontiguousarray(qsh.T).astype(BF16),
            "qst": np.ascontiguousarray(qssh.T).astype(BF16),
            "at": at_b,
            "ast": ast_b,
            "yt": np.ascontiguousarray(y[r0:r1].T),
            "misc": misc,
        }
        for i in range(4):
            m[f"wl{i}"] = wl[i]
        in_maps.append(m)
    return in_maps


_cache = {}
_last_results = None


def _ensure_ntff_hook():
    """Register the axon NTFF profile hook if the image's antenv lacks it."""
    import sys, types, ctypes, contextlib
    try:
        from antenv.axon_hooks import get_axon_ntff_profile_hook  # noqa: F401
        return True
    except ImportError:
        pass
    so_path = "/opt/axon/libaxon_pjrt.so"
    if not os.path.exists(so_path):
        return False
    lib = ctypes.CDLL(so_path)
    if not hasattr(lib, "axon_start_nrt_profile"):
        return False
    lib.axon_start_nrt_profile.argtypes = [ctypes.POINTER(ctypes.c_int64),
                                           ctypes.c_size_t]
    lib.axon_start_nrt_profile.restype = ctypes.c_int64
    lib.axon_stop_nrt_profile.argtypes = [ctypes.c_char_p]
    lib.axon_stop_nrt_profile.restype = ctypes.c_int64

    @contextlib.contextmanager
    def _hook(output_dir, device_ids):
        import jax
        jax.devices()
        if device_ids:
            ids = (ctypes.c_int64 * len(device_ids))(*device_ids)
            rc = lib.axon_start_nrt_profile(ids, len(device_ids))
        else:
            rc = lib.axon_start_nrt_profile(None, 0)
        if rc != 0:
            raise RuntimeError(f"axon_start_nrt_profile rc={rc}")
        try:
            yield
        finally:
            n = lib.axon_stop_nrt_profile(str(output_dir).encode())
            print(f"profile: {n} file(s) written to {output_dir}",
                  file=sys.stderr)

    mod = types.ModuleType("antenv.axon_hooks")
    holder = [_hook]
    mod.get_axon_ntff_profile_hook = lambda: holder[0]
    mod.set_axon_ntff_profile_hook = lambda h: holder.__setitem__(0, h)
    sys.modules["antenv.axon_hooks"] = mod
    import antenv
    antenv.axon_hooks = mod
    return True


def kernel(**inputs):
    global _last_results
    if "nc" not in _cache:
        _cache["nc"] = build()
    nc = _cache["nc"]
    rows = HW // NCORES
    in_maps = prep_inputs(rows, NB, NS, NCORES, **inputs)
    from concourse.bass_utils import run_bass_kernel_spmd
    trace = bool(os.environ.get("KERNEL_TRACE")) and _ensure_ntff_hook()
    res = run_bass_kernel_spmd(nc, in_maps, core_ids=list(range(NCORES)),
                               trace=trace)
    _last_results = res
    Y = np.concatenate([np.asarray(r["yo"]) for r in res.results], axis=0)
    L = np.concatenate([np.asarray(r["lo"]) for r in res.results], axis=0)
    return Y, L



# revision 17
# speedup vs baseline: 1.2612x; 1.2612x over previous
"""Trainium2 Bass kernel for nn_DSR_GCN (dual-superpixel GCN).

Sharding (8 NeuronCores, SPMD): row-shard the HW=65536 pixel dim (8192
rows/core).  Pass 1 computes per-core partials G.T = x_shard.T @ Qc_shard and
column sums (ones-stationary matmuls sharing the same moving Qc stream), then
AllReduces them in bf16 (big branch early so its GCN overlaps the small
pass-1).  Q/Qsmall are shipped centered (Q-0.5) in fp8-e4m3 and matmuls use
DoubleRow perf mode (2 k-tiles per pass); the exact mean term is restored via
an AllReduced xsum column (pass 1) and a rowsum-of-rp correction (pass 2).
The small [N,N] GCN math is replicated on every core in "transposed land"
(feature-major [F, N] layouts) so BatchNorm/bias are per-partition ops.
Pass 2 computes z.T = RP1.T @ Qc.T + RP2.T @ Qcs.T with the final linear
layers folded into tiny [N,32] fp8 stationaries, transposes 512-row chunks
back to pixel-major via the PE, and runs the softmax/loss epilogue.
"""

import os
import numpy as np
import ml_dtypes

BF16 = ml_dtypes.bfloat16
F8 = ml_dtypes.float8_e4m3

HW, C = 65536, 128
NB, NS, NCLS = 1024, 2048, 16
NCORES = 8
EPS = 1e-5
CLAMP = 0.03


def _emit(nc, tc, ctx, rows, nb, ns, ncores):
    import concourse.bass as bass
    import concourse.mybir as mybir
    from concourse import masks
    from contextlib import ExitStack

    f32 = mybir.dt.float32
    bf16 = mybir.dt.bfloat16
    f8 = mybir.dt.float8e4
    ts = bass.ts
    AF = mybir.ActivationFunctionType
    ALU = mybir.AluOpType
    AX = mybir.AxisListType.X
    DR = mybir.MatmulPerfMode.DoubleRow

    # ---- dram I/O ----
    din = lambda n_, s, d: nc.dram_tensor(n_, s, d, kind="ExternalInput")
    xs = din("xs", [rows, C], bf16)
    q = din("q", [rows, nb], bf16)
    qs = din("qs", [rows, ns], bf16)
    qbt = din("qbt", [nb, rows], f8)
    qst = din("qst", [ns, rows], f8)
    at = din("at", [nb, nb], bf16)
    ast = din("ast", [ns, ns], bf16)
    yt = din("yt", [64, rows], bf16)
    FOS = [128, 64, 128, 64]
    wls = [din(f"wl{i}", [128, 256 + 2 * fo + 5], f32) for i, fo in enumerate(FOS)]
    misc = din("misc", [64, 81], f32)
    yo = nc.dram_tensor("yo", [rows, NCLS], f32, kind="ExternalOutput")
    lo = nc.dram_tensor("lo", [rows, NCLS], f32, kind="ExternalOutput")

    # ---- persistent pools ----
    consts = ctx.enter_context(tc.tile_pool(name="consts", bufs=1))
    gwork = ctx.enter_context(tc.tile_pool(name="gwork", bufs=1))
    dram = ctx.enter_context(tc.tile_pool(name="dram", bufs=1, space="DRAM"))

    ident32 = consts.tile([32, 32], f32)
    masks.make_identity(nc, ident32[:])
    ident128 = consts.tile([128, 128], f32)
    masks.make_identity(nc, ident128[:])
    one_col = consts.tile([128, 1], f32)
    nc.gpsimd.memset(one_col[:], 1.0)
    ident1 = consts.tile([1, 1], f32)
    nc.gpsimd.memset(ident1[:], 1.0)
    ones_k1 = consts.tile([1, 128], f32)
    nc.gpsimd.memset(ones_k1[:], 1.0)
    ones_bf = consts.tile([128, 1], bf16)
    nc.gpsimd.memset(ones_bf[:], 1.0)
    ones_f8 = consts.tile([128, 32], f8)
    nc.gpsimd.memset(ones_f8[:], 1.0)
    eps_c = consts.tile([128, 1], f32)
    nc.gpsimd.memset(eps_c[:], EPS)

    misc_sb = consts.tile([64, 81], f32)
    nc.gpsimd.dma_start(misc_sb[:], misc[:])
    w128bT_bf = consts.tile([64, 16], bf16)
    nc.vector.tensor_copy(w128bT_bf[:], misc_sb[:, 64:80])
    wl_sb = []
    for i, fo in enumerate(FOS):
        t = consts.tile([128, 256 + 2 * fo + 5], f32, tag=f"wl{i}")
        nc.gpsimd.dma_start(t[:], wls[i][:])
        wl_sb.append(t)

    # ---- pass 1 (bf16; colsums via DVE accumulation, no ones-matmuls) ----
    n_rt = rows // 128
    shkw = {"addr_space": "Shared"} if ncores > 4 else {}
    ar1_in = dram.tile([129, nb], f32, tag="ar1i")
    ar1_out = dram.tile([129, nb], f32, tag="ar1o", **shkw)
    ar2_in = dram.tile([129, ns], f32, tag="ar2i")
    ar2_out = dram.tile([129, ns], f32, tag="ar2o", **shkw)

    with tc.tile_pool(name="p1pool", bufs=1) as p1pool:
        xall = p1pool.tile([128, n_rt * C], bf16, tag="xall")
        nc.gpsimd.dma_start(
            xall[:].rearrange("p (t c) -> p t c", c=C),
            xs[:].rearrange("(t p) c -> p t c", p=128))

        def pass1_phase(qd, n, g_ps, cs_ps, rgrp, qtag, qpool, acc):
            for g in range(n_rt // rgrp):
                qt = qpool.tile([128, rgrp * n], bf16, tag=qtag)
                for a in range(rgrp):
                    rt = g * rgrp + a
                    nc.gpsimd.dma_start(qt[:, a * n:(a + 1) * n],
                                        qd[rt * 128:(rt + 1) * 128, :])
                for a in range(rgrp):
                    rt = g * rgrp + a
                    xt = xall[:, ts(rt, C)]
                    st = (rt == 0)
                    sp = (rt == n_rt - 1)
                    for cnk in range(n // 512):
                        mv = qt[:, a * n + cnk * 512:a * n + (cnk + 1) * 512]
                        nc.tensor.matmul(g_ps[:, ts(cnk, 512)], xt, mv,
                                         start=st, stop=sp)
                    if rt == 0:
                        nc.vector.tensor_copy(acc[:], qt[:, 0:n])
                    else:
                        nc.vector.tensor_tensor(
                            acc[:], acc[:], qt[:, a * n:(a + 1) * n],
                            op=ALU.add)
            for cnk in range(n // 512):
                nc.tensor.matmul(cs_ps[:, ts(cnk, 512)], ones_bf[:],
                                 acc[:, ts(cnk, 512)], start=True, stop=True)

        with tc.tile_pool(name="ps_p1b", bufs=1, space="PSUM") as psb, \
             tc.tile_pool(name="qpb", bufs=3) as qpool:
            g1p = psb.tile([128, nb], f32, tag="g1p")
            cs1p = psb.tile([1, nb], f32, tag="cs1p")
            acc1 = p1pool.tile([128, nb], bf16, tag="acc1")
            pass1_phase(q, nb, g1p, cs1p, min(4096 // nb, n_rt), "qb", qpool,
                        acc1)
            g1t = p1pool.tile([128, nb], f32, tag="g1t")
            cs1 = p1pool.tile([1, nb], f32, tag="cs1")
            nc.vector.tensor_copy(g1t[:], g1p[:])
            nc.vector.tensor_copy(cs1[:], cs1p[:])

        # big-branch AllReduce early: overlaps small pass-1
        nc.gpsimd.dma_start(ar1_in[0:128, :], g1t[:])
        nc.gpsimd.dma_start(ar1_in[128:129, :], cs1[:])
        nc.gpsimd.collective_compute(
            "AllReduce", mybir.AluOpType.add,
            replica_groups=[list(range(ncores))],
            ins=[ar1_in.opt()], outs=[ar1_out.opt()])

        with tc.tile_pool(name="ps_p1s", bufs=1, space="PSUM") as pss, \
             tc.tile_pool(name="qps", bufs=3) as qpool:
            g2p = pss.tile([128, ns], f32, tag="g2p")
            cs2p = pss.tile([1, ns], f32, tag="cs2p")
            acc2 = p1pool.tile([128, ns], bf16, tag="acc2")
            pass1_phase(qs, ns, g2p, cs2p, min(4096 // ns, n_rt), "qs", qpool,
                        acc2)
            g2t = p1pool.tile([128, ns], f32, tag="g2t")
            cs2 = p1pool.tile([1, ns], f32, tag="cs2")
            nc.vector.tensor_copy(g2t[:], g2p[:])
            nc.vector.tensor_copy(cs2[:], cs2p[:])

        nc.gpsimd.dma_start(ar2_in[0:128, :], g2t[:])
        nc.gpsimd.dma_start(ar2_in[128:129, :], cs2[:])
        nc.gpsimd.collective_compute(
            "AllReduce", mybir.AluOpType.add,
            replica_groups=[list(range(ncores))],
            ins=[ar2_in.opt()], outs=[ar2_out.opt()])

    # ---- GCN (replicated per core) ----
    def gcn_branch(n, ar_out, at_d, lidx, clamp, btag, hfin):
        njt = n // 128
        ncnk = n // 512
        with ExitStack() as bctx:
            bp = bctx.enter_context(tc.tile_pool(name=f"b_{btag}", bufs=1))

            ht = bp.tile([128, n], f32, tag="hcur0")
            with tc.tile_pool(name=f"psr_{btag}", bufs=2, space="PSUM") as psr, \
                 tc.tile_pool(name=f"icsp_{btag}", bufs=1) as icsp:
                g_sb = icsp.tile([128, n], f32, tag="g_sb")
                cs_sb = icsp.tile([1, n], f32, tag="cs_sb")
                nc.gpsimd.dma_start(g_sb[:], ar_out[0:128, 0:n])
                nc.gpsimd.dma_start(cs_sb[:], ar_out[128:129, 0:n])
                ics = icsp.tile([1, n], f32, tag="ics")
                nc.vector.reciprocal(ics[:], cs_sb[:])
                for cnk in range(ncnk):
                    pr = psr.tile([128, 512], f32)
                    nc.tensor.matmul(pr[:], ones_k1[:],
                                     ics[:, ts(cnk, 512)],
                                     start=True, stop=True)
                    nc.vector.tensor_tensor(
                        ht[:, ts(cnk, 512)], g_sb[:, ts(cnk, 512)], pr[:],
                        op=ALU.mult)

            for li2, wli in enumerate(lidx):
                fo = FOS[wli]
                wl = wl_sb[wli]
                last = (li2 == 1)
                c0 = 256 + fo
                thWT = wl[:, 0:256]
                oWT = wl[:, 256:256 + fo]
                bng = wl[:, c0:c0 + 1]
                bnb = wl[:, c0 + 1:c0 + 2]
                thb = [wl[:, c0 + 2:c0 + 3], wl[:, c0 + 3:c0 + 4]]
                ob_col = wl[0:fo, c0 + 4:c0 + 5]
                ob_row = wl[0:1, c0 + 5:c0 + 5 + fo]

                with ExitStack() as lctx:
                    lp = lctx.enter_context(
                        tc.tile_pool(name=f"l_{btag}{li2}", bufs=1))
                    sp = lctx.enter_context(
                        tc.tile_pool(name=f"sp_{btag}{li2}", bufs=2))

                    # --- batchnorm over nodes (free dim) ---
                    s1 = sp.tile([128, 1], f32, tag="s1")
                    nc.vector.reduce_sum(out=s1[:], in_=ht[:], axis=AX)
                    s2p = sp.tile([128, ncnk], f32, tag="s2p")
                    with tc.tile_pool(name=f"psbn_{btag}{li2}", bufs=2,
                                      space="PSUM") as psbn:
                        for cnk in range(ncnk):
                            pb = psbn.tile([128, 512], f32)
                            nc.scalar.activation(
                                pb[:], ht[:, ts(cnk, 512)], AF.Square,
                                accum_out=s2p[:, cnk:cnk + 1])
                    s2 = sp.tile([128, 1], f32, tag="s2")
                    nc.vector.reduce_sum(out=s2[:], in_=s2p[:], axis=AX)
                    m = sp.tile([128, 1], f32, tag="m")
                    nc.vector.tensor_scalar_mul(m[:], s1[:], 1.0 / n)
                    v = sp.tile([128, 1], f32, tag="v")
                    nc.vector.tensor_scalar_mul(v[:], s2[:], 1.0 / n)
                    m2 = sp.tile([128, 1], f32, tag="m2")
                    nc.vector.tensor_tensor(m2[:], m[:], m[:], op=ALU.mult)
                    nc.vector.tensor_tensor(v[:], v[:], m2[:], op=ALU.subtract)
                    sd = sp.tile([128, 1], f32, tag="sd")
                    nc.scalar.activation(sd[:], v[:], AF.Sqrt, bias=eps_c[:])
                    isd = sp.tile([128, 1], f32, tag="isd")
                    nc.vector.reciprocal(isd[:], sd[:])
                    kk = sp.tile([128, 1], f32, tag="kk")
                    nc.vector.tensor_tensor(kk[:], bng, isd[:], op=ALU.mult)
                    b2 = sp.tile([128, 1], f32, tag="b2")
                    nc.vector.tensor_tensor(b2[:], m[:], kk[:], op=ALU.mult)
                    nc.vector.tensor_tensor(b2[:], bnb, b2[:], op=ALU.subtract)
                    hbn = lp.tile([128, n], bf16, tag="hbn")
                    nc.vector.tensor_scalar(hbn[:], ht[:], kk[:], b2[:],
                                            op0=ALU.mult, op1=ALU.add)
                    thWT_bf = lp.tile([128, 256], bf16, tag="thWT_bf")
                    nc.vector.tensor_copy(thWT_bf[:], thWT)
                    oWT_bf = lp.tile([128, fo], bf16, tag="oWT_bf")
                    nc.vector.tensor_copy(oWT_bf[:], oWT)

                    # --- Hx.T = thW @ Hbn.T + thb ---
                    hx = [lp.tile([128, n], bf16, tag=f"hx{k}", name=f"hx{k}")
                          for k in range(2)]
                    with tc.tile_pool(name=f"psx_{btag}{li2}", bufs=3,
                                      space="PSUM") as psx:
                        for k in range(2):
                            for cnk in range(ncnk):
                                px = psx.tile([128, 512], f32)
                                nc.tensor.matmul(
                                    px[:], thWT_bf[:, ts(k, 128)],
                                    hbn[:, ts(cnk, 512)],
                                    start=True, stop=True)
                                nc.vector.tensor_scalar_add(
                                    hx[k][:, ts(cnk, 512)], px[:], thb[k])

                    # --- S blocks -> sigmoid -> t = S'*A.T ; d_pre ---
                    tt = []
                    thr = float(np.log(CLAMP / (1.0 - CLAMP)))
                    with tc.tile_pool(name=f"psd_{btag}{li2}", bufs=1,
                                      space="PSUM") as psd, \
                         tc.tile_pool(name=f"pss_{btag}{li2}", bufs=3,
                                      space="PSUM") as pssb, \
                         tc.tile_pool(name=f"atp_{btag}{li2}", bufs=2) as atp:
                        dpre_ps = psd.tile([1, n], f32, tag="dpre")
                        for j in range(njt):
                            sbl = sp.tile([128, n], bf16, tag="sblk")
                            for cnk in range(ncnk):
                                px = pssb.tile([128, 512], f32)
                                nc.tensor.matmul(px[:],
                                                 hx[0][:, ts(j, 128)],
                                                 hx[0][:, ts(cnk, 512)],
                                                 start=True, stop=False)
                                nc.tensor.matmul(px[:],
                                                 hx[1][:, ts(j, 128)],
                                                 hx[1][:, ts(cnk, 512)],
                                                 start=False, stop=True)
                                if clamp:
                                    xc = sp.tile([128, 512], f32, tag="xc")
                                    nc.vector.tensor_scalar_max(
                                        xc[:], px[:], thr)
                                    nc.scalar.activation(
                                        sbl[:, ts(cnk, 512)], xc[:], AF.Sigmoid)
                                else:
                                    nc.scalar.activation(
                                        sbl[:, ts(cnk, 512)], px[:], AF.Sigmoid)
                            att = atp.tile([128, n], bf16)
                            nc.gpsimd.dma_start(att[:], at_d[ts(j, 128), :])
                            tj = lp.tile([128, n], bf16, tag=f"tj{j}")
                            nc.vector.tensor_tensor(tj[:], sbl[:], att[:],
                                                    op=ALU.mult)
                            tt.append(tj)
                            for cnk in range(ncnk):
                                nc.tensor.matmul(
                                    dpre_ps[:, ts(cnk, 512)], ones_bf[:],
                                    tj[:, ts(cnk, 512)],
                                    start=(j == 0), stop=(j == njt - 1))
                        dpre = lp.tile([1, n], f32, tag="dpre_sb")
                        nc.vector.tensor_copy(dpre[:], dpre_ps[:])

                    # column-form d = (dpre + 1)^-1/2 (fast [128, njt] math)
                    dcol = sp.tile([128, njt], f32, tag="dcol")
                    drow = lp.tile([1, n], f32, tag="drow")
                    ob_rep = sp.tile([128, fo], f32, tag="ob_rep")
                    with tc.tile_pool(name=f"pst_{btag}{li2}", bufs=2,
                                      space="PSUM") as pst:
                        draw = sp.tile([128, njt], f32, tag="draw")
                        for j in range(njt):
                            pt = pst.tile([128, 1], f32, tag="dt")
                            nc.tensor.transpose(pt[:], dpre[:, ts(j, 128)],
                                                ident1[:])
                            nc.vector.tensor_copy(draw[:, j:j + 1], pt[:])
                        nc.scalar.activation(draw[:], draw[:], AF.Sqrt,
                                             bias=one_col[:])
                        nc.vector.reciprocal(dcol[:], draw[:])
                        for j in range(njt):
                            pt2 = pst.tile([1, 128], f32, tag="dt2")
                            nc.tensor.transpose(pt2[:], dcol[:, j:j + 1],
                                                ident128[:])
                            nc.vector.tensor_copy(drow[:, ts(j, 128)], pt2[:])
                        pr = pst.tile([128, fo], f32, tag="obr")
                        nc.tensor.matmul(pr[:], ones_k1[:], ob_row,
                                         start=True, stop=True)
                        nc.vector.tensor_copy(ob_rep[:], pr[:])

                    # stat_j = d_j * (HoW_j + ob)   [128, fo] bf16
                    stats = []
                    with tc.tile_pool(name=f"psh_{btag}{li2}", bufs=3,
                                      space="PSUM") as psh:
                        for j in range(njt):
                            ph = psh.tile([128, fo], f32)
                            nc.tensor.matmul(ph[:], hbn[:, ts(j, 128)],
                                             oWT_bf[:], start=True, stop=True)
                            w1 = sp.tile([128, fo], f32, tag="w1")
                            nc.vector.tensor_tensor(w1[:], ph[:], ob_rep[:],
                                                    op=ALU.add)
                            stj = lp.tile([128, fo], bf16, tag=f"st{j}")
                            nc.vector.tensor_scalar_mul(stj[:], w1[:],
                                                        dcol[:, j:j + 1])
                            stats.append(stj)

                    # u = HoW.T + ob
                    u = lp.tile([fo, n], f32, tag="u")
                    with tc.tile_pool(name=f"psu_{btag}{li2}", bufs=2,
                                      space="PSUM") as psu:
                        for cnk in range(ncnk):
                            pu = psu.tile([fo, 512], f32)
                            nc.tensor.matmul(pu[:], oWT_bf[:],
                                             hbn[:, ts(cnk, 512)],
                                             start=True, stop=True)
                            nc.vector.tensor_scalar_add(
                                u[:, ts(cnk, 512)], pu[:], ob_col)

                    # out.T = d ⊙ (stat.T @ t + d ⊙ u); leaky relu
                    hnext = hfin if last else bp.tile([128, n], f32,
                                                      tag="hcur1")
                    with tc.tile_pool(name=f"pso_{btag}{li2}", bufs=1,
                                      space="PSUM") as pso, \
                         tc.tile_pool(name=f"psq_{btag}{li2}", bufs=2,
                                      space="PSUM") as psq:
                        po = pso.tile([fo, n], f32, tag="po")
                        for j in range(njt):
                            for cnk in range(ncnk):
                                nc.tensor.matmul(po[:, ts(cnk, 512)],
                                                 stats[j][:],
                                                 tt[j][:, ts(cnk, 512)],
                                                 start=(j == 0),
                                                 stop=(j == njt - 1))
                        for cnk in range(ncnk):
                            pr = psq.tile([fo, 512], f32)
                            nc.tensor.matmul(pr[:], ones_k1[:, 0:fo],
                                             drow[:, ts(cnk, 512)],
                                             start=True, stop=True)
                            z1 = sp.tile([fo, 512], f32, tag="z1")
                            nc.vector.tensor_tensor(
                                z1[:], u[:, ts(cnk, 512)], pr[:], op=ALU.mult)
                            vv = sp.tile([fo, 512], f32, tag="vv")
                            nc.vector.tensor_tensor(
                                vv[:], po[:, ts(cnk, 512)], z1[:], op=ALU.add)
                            nc.vector.tensor_tensor(vv[:], vv[:], pr[:],
                                                    op=ALU.mult)
                            lk = sp.tile([fo, 512], f32, tag="lk")
                            nc.vector.tensor_scalar_mul(lk[:], vv[:], 0.01)
                            nc.vector.tensor_tensor(
                                hnext[0:fo, ts(cnk, 512)], vv[:], lk[:],
                                op=ALU.max)
                ht = hnext

    h1f = gwork.tile([64, nb], f32, tag="h1f")
    h2f = gwork.tile([64, ns], f32, tag="h2f")
    gcn_branch(nb, ar1_out, at, [0, 1], True, "big", h1f)
    gcn_branch(ns, ar2_out, ast, [2, 3], False, "sml", h2f)

    # RP stationaries [spix, 32] fp8, final linears folded
    wcb = misc_sb[:, 0:32]
    wcs = misc_sb[:, 32:64]
    b128 = misc_sb[0:16, 80:81]
    rp1 = gwork.tile([128, (nb // 128) * 32], f8, tag="rp1")
    rp2 = gwork.tile([128, (ns // 128) * 32], f8, tag="rp2")
    with tc.tile_pool(name="psrp", bufs=3, space="PSUM") as psrp:
        for j in range(nb // 128):
            pr = psrp.tile([128, 32], f32)
            nc.tensor.matmul(pr[:], h1f[:, ts(j, 128)], wcb,
                             start=True, stop=True)
            nc.vector.tensor_copy(rp1[:, ts(j, 32)], pr[:])
        for j in range(ns // 128):
            pr = psrp.tile([128, 32], f32)
            nc.tensor.matmul(pr[:], h2f[:, ts(j, 128)], wcs,
                             start=True, stop=True)
            nc.vector.tensor_copy(rp2[:, ts(j, 32)], pr[:])

    # centered-q correction: c32 = sum_n rp[n, :], bias_col = [b128;0]+0.5*c32
    bias_col = gwork.tile([32, 1], f32, tag="bias_col")
    with tc.tile_pool(name="psc", bufs=2, space="PSUM") as psc, \
         tc.tile_pool(name="cwk", bufs=1) as cwk:
        c32p = psc.tile([1, 32], f32, tag="c32p")
        ovp = ones_f8[:].rearrange("p (a o) -> p a o", o=16)[:, :, 0:1]
        r1v = rp1[:].rearrange("p (j f) -> p j f", f=32)
        r2v = rp2[:].rearrange("p (j f) -> p j f", f=32)
        njb, njs = nb // 128, ns // 128
        for jp in range(njb // 2):
            nc.tensor.matmul(c32p[:], ovp, r1v[:, 2 * jp:2 * jp + 2, :],
                             start=(jp == 0), stop=False, perf_mode=DR)
        for jp in range(njs // 2):
            nc.tensor.matmul(c32p[:], ovp, r2v[:, 2 * jp:2 * jp + 2, :],
                             start=False, stop=(jp == njs // 2 - 1),
                             perf_mode=DR)
        c32s = cwk.tile([1, 32], f32, tag="c32s")
        nc.vector.tensor_copy(c32s[:], c32p[:])
        c32t_p = psc.tile([32, 1], f32, tag="c32tp")
        nc.tensor.transpose(c32t_p[:], c32s[:], ident1[:])
        bext = cwk.tile([32, 1], f32, tag="bext")
        nc.gpsimd.memset(bext[:], 0.0)
        nc.vector.tensor_copy(bext[0:16, :], b128)
        nc.vector.scalar_tensor_tensor(
            bias_col[:], c32t_p[:], 0.5, bext[:], op0=ALU.mult, op1=ALU.add)

    # ---- pass 2 + epilogue (fp8 DoubleRow over superpixel pairs) ----
    GRP = min(2048, rows)
    nrc = max(GRP // 512, 1)
    CH = GRP // nrc
    with tc.tile_pool(name="qtp", bufs=2) as qtp, \
         tc.tile_pool(name="ytp", bufs=2) as ytp, \
         tc.tile_pool(name="ps_z", bufs=4, space="PSUM") as ps_z, \
         tc.tile_pool(name="ps_yw", bufs=2, space="PSUM") as ps_yw, \
         tc.tile_pool(name="ps_tp", bufs=2, space="PSUM") as ps_tp, \
         tc.tile_pool(name="epil", bufs=4) as ep:
        r1v = rp1[:].rearrange("p (j f) -> p j f", f=32)
        r2v = rp2[:].rearrange("p (j f) -> p j f", f=32)
        njb, njs = nb // 128, ns // 128
        for gidx in range(rows // GRP):
            ytt = ytp.tile([64, GRP], bf16, tag="ytt")
            nc.gpsimd.dma_start(ytt[:], yt[:, gidx * GRP:(gidx + 1) * GRP])
            pz = [ps_z.tile([32, CH], f32, tag="pz", name=f"pz{gidx}_{i}")
                  for i in range(nrc)]

            def pz_accum(qtd, rv, j0, jcnt, qtag, first, last):
                # one slab DMA for jcnt j-tiles, DoubleRow over j pairs
                tq = qtp.tile([128, jcnt * GRP], f8, tag=qtag)
                nc.gpsimd.dma_start(
                    tq[:].rearrange("p (j g) -> p j g", j=jcnt),
                    qtd[j0 * 128:(j0 + jcnt) * 128,
                        gidx * GRP:(gidx + 1) * GRP].rearrange(
                            "(j p) g -> p j g", p=128))
                tqv = tq[:].rearrange("p (j g) -> p j g", j=jcnt)
                for jp in range(jcnt // 2):
                    stat = rv[:, j0 + 2 * jp:j0 + 2 * jp + 2, :]
                    for rc in range(nrc):
                        nc.tensor.matmul(
                            pz[rc][:], stat,
                            tqv[:, 2 * jp:2 * jp + 2, ts(rc, CH)],
                            start=(first and jp == 0),
                            stop=(last and jp == jcnt // 2 - 1), perf_mode=DR)

            pz_accum(qbt, r1v, 0, njb, "tqb", True, False)
            pz_accum(qst, r2v, 0, njs // 2, "tqs0", False, False)
            pz_accum(qst, r2v, njs // 2, njs // 2, "tqs1", False, True)
            for rc in range(nrc):
                base = gidx * GRP + rc * CH
                nsc = CH // 128
                pyw = ps_yw.tile([16, CH], f32)
                nc.tensor.matmul(pyw[:], w128bT_bf[:], ytt[:, ts(rc, CH)],
                                 start=True, stop=True)
                yws = ep.tile([16, CH], f32, tag="yws")
                nc.scalar.activation(yws[:], pyw[:], AF.Copy)
                tri = ep.tile([32, CH], f32, tag="tri")
                nc.vector.tensor_scalar_add(tri[:], pz[rc][:], bias_col[:])
                nc.vector.tensor_tensor(tri[0:16, :], tri[0:16, :], yws[:],
                                        op=ALU.add)
                yac = ep.tile([128, nsc * 16], f32, tag="yac")
                lac = ep.tile([128, nsc * 16], f32, tag="lac")
                for s in range(nsc):
                    ptr = ps_tp.tile([128, 32], f32)
                    nc.tensor.transpose(ptr[:], tri[:, ts(s, 128)], ident32[:])
                    mx = ep.tile([128, 1], f32, tag="mx")
                    nc.vector.reduce_max(out=mx[:], in_=ptr[:, 0:16], axis=AX)
                    nmx = ep.tile([128, 1], f32, tag="nmx")
                    nc.vector.tensor_scalar_mul(nmx[:], mx[:], -1.0)
                    e = ep.tile([128, 16], f32, tag="e")
                    ssum = ep.tile([128, 1], f32, tag="ssum")
                    nc.scalar.activation(e[:], ptr[:, 0:16], AF.Exp,
                                         bias=nmx[:], accum_out=ssum[:])
                    rcp = ep.tile([128, 1], f32, tag="rcp")
                    nc.vector.reciprocal(rcp[:], ssum[:])
                    nc.vector.tensor_scalar_mul(yac[:, ts(s, 16)], e[:],
                                                rcp[:])
                    nc.scalar.activation(lac[:, ts(s, 16)], ptr[:, 16:32],
                                         AF.Square)
                nc.gpsimd.dma_start(
                    yo[base:base + CH, :].rearrange("(s p) f -> p s f", p=128),
                    yac[:].rearrange("p (s f) -> p s f", s=nsc))
                nc.gpsimd.dma_start(
                    lo[base:base + CH, :].rearrange("(s p) f -> p s f", p=128),
                    lac[:].rearrange("p (s f) -> p s f", s=nsc))


def build(rows=HW // NCORES, nb=NB, ns=NS, ncores=NCORES):
    from contextlib import ExitStack
    import concourse.bacc as bacc
    import concourse.tile as tile

    nc = bacc.Bacc("TRN2", target_bir_lowering=False, debug=False,
                   enable_asserts=True, num_devices=ncores)
    with tile.TileContext(nc) as tc:
        with ExitStack() as ctx:
            _emit(nc, tc, ctx, rows, nb, ns, ncores)
    nc.compile()
    return nc


# --------------------------------------------------------------------------
# host wrapper
# --------------------------------------------------------------------------

def prep_inputs(rows, nb, ns, ncores,
                x, y, Q, A, Qsmall, Asmall,
                b0_bng, b0_bnb, b0_thW, b0_thb, b0_oW, b0_ob,
                b1_bng, b1_bnb, b1_thW, b1_thb, b1_oW, b1_ob,
                s0_bng, s0_bnb, s0_thW, s0_thb, s0_oW, s0_ob,
                s1_bng, s1_bnb, s1_thW, s1_thb, s1_oW, s1_ob,
                lin128_W, lin128_b, lin64_W, lin64_b, sigma2):
    f = np.float32
    hw = rows * ncores
    flat = np.ascontiguousarray(np.asarray(x, f).reshape(hw, -1))
    Qf = np.asarray(Q, f)
    Qsf = np.asarray(Qsmall, f)
    y = np.asarray(y, f)

    def wl_pack(thW, thb, oW, ob, bng, bnb):
        fo = np.asarray(oW).shape[0]
        w = np.zeros((128, 256 + 2 * fo + 5), f)
        w[:, 0:256] = np.asarray(thW, f).T
        w[:, 256:256 + fo] = np.asarray(oW, f).T
        c0 = 256 + fo
        w[:, c0] = np.asarray(bng, f)
        w[:, c0 + 1] = np.asarray(bnb, f)
        w[:, c0 + 2] = np.asarray(thb, f)[0:128]
        w[:, c0 + 3] = np.asarray(thb, f)[128:256]
        w[0:fo, c0 + 4] = np.asarray(ob, f)
        w[0, c0 + 5:c0 + 5 + fo] = np.asarray(ob, f)
        return w

    wl = [
        wl_pack(b0_thW, b0_thb, b0_oW, b0_ob, b0_bng, b0_bnb),
        wl_pack(b1_thW, b1_thb, b1_oW, b1_ob, b1_bng, b1_bnb),
        wl_pack(s0_thW, s0_thb, s0_oW, s0_ob, s0_bng, s0_bnb),
        wl_pack(s1_thW, s1_thb, s1_oW, s1_ob, s1_bng, s1_bnb),
    ]

    sig = float(np.asarray(sigma2).reshape(-1)[0])
    W128 = np.asarray(lin128_W, f)
    W64 = np.asarray(lin64_W, f)
    misc = np.zeros((64, 81), f)
    misc[:, 0:16] = sig * W128[:, :64].T
    misc[:, 16:32] = W64.T
    misc[:, 32:48] = (1.0 - sig) * W128[:, :64].T
    misc[:, 48:64] = -W64.T
    misc[:, 64:80] = W128[:, 64:].T
    misc[0:16, 80] = np.asarray(lin128_b, f)

    at_b = np.ascontiguousarray(np.asarray(A, f).T).astype(BF16)
    ast_b = np.ascontiguousarray(np.asarray(Asmall, f).T).astype(BF16)

    in_maps = []
    for c in range(ncores):
        r0, r1 = c * rows, (c + 1) * rows
        qsh = Qf[r0:r1]
        qssh = Qsf[r0:r1]
        m = {
            "xs": flat[r0:r1].astype(BF16),
            "q": qsh.astype(BF16),
            "qs": qssh.astype(BF16),
            "qbt": np.ascontiguousarray(qsh.T - 0.5).astype(F8),
            "qst": np.ascontiguousarray(qssh.T - 0.5).astype(F8),
            "at": at_b,
            "ast": ast_b,
            "yt": np.ascontiguousarray(y[r0:r1].T).astype(BF16),
            "misc": misc,
        }
        for i in range(4):
            m[f"wl{i}"] = wl[i]
        in_maps.append(m)
    return in_maps


_cache = {}
_last_results = None


def _ensure_ntff_hook():
    """Register the axon NTFF profile hook if the image's antenv lacks it."""
    import sys, types, ctypes, contextlib
    try:
        from antenv.axon_hooks import get_axon_ntff_profile_hook  # noqa: F401
        return True
    except ImportError:
        pass
    so_path = "/opt/axon/libaxon_pjrt.so"
    if not os.path.exists(so_path):
        return False
    lib = ctypes.CDLL(so_path)
    if not hasattr(lib, "axon_start_nrt_profile"):
        return False
    lib.axon_start_nrt_profile.argtypes = [ctypes.POINTER(ctypes.c_int64),
                                           ctypes.c_size_t]
    lib.axon_start_nrt_profile.restype = ctypes.c_int64
    lib.axon_stop_nrt_profile.argtypes = [ctypes.c_char_p]
    lib.axon_stop_nrt_profile.restype = ctypes.c_int64

    @contextlib.contextmanager
    def _hook(output_dir, device_ids):
        import jax
        jax.devices()
        if device_ids:
            ids = (ctypes.c_int64 * len(device_ids))(*device_ids)
            rc = lib.axon_start_nrt_profile(ids, len(device_ids))
        else:
            rc = lib.axon_start_nrt_profile(None, 0)
        if rc != 0:
            raise RuntimeError(f"axon_start_nrt_profile rc={rc}")
        try:
            yield
        finally:
            n = lib.axon_stop_nrt_profile(str(output_dir).encode())
            print(f"profile: {n} file(s) written to {output_dir}",
                  file=sys.stderr)

    mod = types.ModuleType("antenv.axon_hooks")
    holder = [_hook]
    mod.get_axon_ntff_profile_hook = lambda: holder[0]
    mod.set_axon_ntff_profile_hook = lambda h: holder.__setitem__(0, h)
    sys.modules["antenv.axon_hooks"] = mod
    import antenv
    antenv.axon_hooks = mod
    return True


def kernel(**inputs):
    global _last_results
    if "nc" not in _cache:
        _cache["nc"] = build()
    nc = _cache["nc"]
    rows = HW // NCORES
    in_maps = prep_inputs(rows, NB, NS, NCORES, **inputs)
    from concourse.bass_utils import run_bass_kernel_spmd
    trace = bool(os.environ.get("KERNEL_TRACE")) and _ensure_ntff_hook()
    res = run_bass_kernel_spmd(nc, in_maps, core_ids=list(range(NCORES)),
                               trace=trace)
    _last_results = res
    Y = np.concatenate([np.asarray(r["yo"]) for r in res.results], axis=0)
    L = np.concatenate([np.asarray(r["lo"]) for r in res.results], axis=0)
    return Y, L
